# revision 26
# baseline (speedup 1.0000x reference)
"""GCNConvSC (residual + GCNConv) Trainium2 Bass kernel, 8-core SPMD.

Math (matches the PyG-style reference):
    deg[v]  = indeg_with_selfloop(v)          (count of v in dst, +1)
    u       = deg^{-1/2}
    y       = u[:,None] * x                   (pre-scaled node features, fp8)
    z[v]    = sum_{e: dst_e = v} y[src_e] * u[v]   (via one-hot matmuls)
    out[v]  = x[v] + b + (z[v] + u[v]^2 * x[v]) @ W

Pipeline per core (dst nodes range-partitioned, S=12544 slots, 98 windows
of 128):
  - y stored in HBM as fp8 e4m3 rows padded to a 256B stride; per-edge rows
    are fetched with a raw InstDMAGatherAnt (elem_size=128, elem_step=256),
    i.e. 128B descriptors, which the DMA cost model prices at half the
    256B-descriptor rate.  Edges are bucketed by (window-group, src-chunk,
    window) with int16 chunk-local indices (4 chunks of 25024 rows).
  - Aggregation: per 128-edge tile a bf16 one-hot (iota==slot)*u[dst] is
    built on DVE (4x perf mode) and matmul'd (fp8 lhsT x bf16 rhs) into a
    PSUM bank quarter for the edge's dst window.
  - The self-loop term ys = u^2*x and the residual xs = x + b are seeded
    into PSUM by identity-rhs matmuls (lhsT = node-major bf16 tiles), so
    the SBUF accumulator is write-once and flushes are plain Activation-
    engine PSUM->SBUF copies (DVE stays free for one-hots).
  - Tail: out^T = W^T @ acc accumulated on top of the xs seed, copied to
    bf16 and stored.
"""

import sys

sys.path.insert(0, "/opt/trn_rl_repo")

import numpy as np

N_NODES = 100000
F = 128
N_CORES = 8
S = 12544            # dst slots per core (98 windows of 128)
WN = 98              # windows per core
WG = 16              # windows per PSUM group (4 banks of 4 windows)
N_CHUNKS = 4
CHUNK = 25024        # gather-source rows per chunk (int16-safe)
NPAD = N_CHUNKS * CHUNK  # 100096 padded node rows for y
YSTRIDE = 256        # fp8 row stride in bytes (DMA desc stride granularity)


def _host_plan(edge_index):
    """Sort/bucket edges per core; emit the shared SPMD schedule plus
    per-core gather-index and slot arrays."""
    src = np.asarray(edge_index[0], dtype=np.int64)
    dst = np.asarray(edge_index[1], dtype=np.int64)

    deg_e = np.bincount(dst, minlength=N_NODES)
    u = (1.0 / np.sqrt(deg_e.astype(np.float64) + 1.0)).astype(np.float32)

    chunk_of = src // CHUNK

    # Window-classes of 1024 similar-degree dsts (descending degree); within
    # each class, greedily deal the dsts to the 8 cores balancing the
    # per-chunk edge-count vectors, so the shared max-over-cores schedule
    # pads as little as possible.
    dvec = np.zeros((N_NODES, N_CHUNKS), np.int64)
    np.add.at(dvec, (dst, chunk_of), 1)
    order = np.argsort(-deg_e, kind="stable")

    perm = np.full((N_CORES, S), -1, dtype=np.int64)
    core_of_node = np.empty(N_NODES, dtype=np.int64)
    pos_of_node = np.empty(N_NODES, dtype=np.int64)
    inf = np.float64(1e18)
    for w in range(WN):
        cls = order[w * 128 * N_CORES : (w + 1) * 128 * N_CORES]
        L = np.zeros((N_CORES, N_CHUNKS), np.float64)
        cap = np.full(N_CORES, 128, np.int64)
        nxt = np.full(N_CORES, w * 128, np.int64)
        for v in cls:
            cost = ((L + dvec[v]) ** 2).sum(axis=1)
            cost[cap == 0] = inf
            c = int(np.argmin(cost))
            L[c] += dvec[v]
            cap[c] -= 1
            perm[c, nxt[c]] = v
            core_of_node[v] = c
            pos_of_node[v] = nxt[c]
            nxt[c] += 1

    core_of = core_of_node[dst]
    pos_e_all = pos_of_node[dst]
    u_e_all = u[dst]

    per_core = []
    counts = np.zeros((N_CORES, N_CHUNKS, WN), dtype=np.int64)
    for c in range(N_CORES):
        m = core_of == c
        es, pos_e, ue = src[m], pos_e_all[m], u_e_all[m]
        ch = chunk_of[m]
        w = pos_e // 128
        slot = pos_e % 128
        wg = w // WG
        so = np.lexsort((w, ch, wg))
        es, slot, ch, w, ue = es[so], slot[so], ch[so], w[so], ue[so]
        np.add.at(counts[c], (ch, w), 1)
        per_core.append((es, slot, ch, w, ue))

    # shared schedule: tiles per (chunk, window) = max over cores; windows
    # with zero edges need no tiles (their PSUM quarter is seeded anyway)
    n_tiles = (counts.max(axis=0) + 127) // 128  # [N_CHUNKS, WN]

    n_wg = (WN + WG - 1) // WG
    sched = []  # (g, segs) with segs[ch] = [(window, ntiles), ...]
    T = 0
    for g in range(n_wg):
        ws = range(g * WG, min((g + 1) * WG, WN))
        segs = []
        for ch in range(N_CHUNKS):
            tl = [(w, int(n_tiles[ch, w])) for w in ws if n_tiles[ch, w] > 0]
            segs.append(tl)
        sched.append((g, segs))
        T += int(n_tiles[:, list(ws)].sum())

    # per-core padded edge streams in schedule order
    idx16 = np.zeros((N_CORES, T * 128), dtype=np.int16)
    slots = np.full((N_CORES, T * 128), -1.0, dtype=np.float32)
    uvals = np.zeros((N_CORES, T * 128), dtype=np.float32)
    for c in range(N_CORES):
        es, eslot, ch, w, ue = per_core[c]
        keys = list(zip(w // WG, ch, w))
        run_start = {}
        for i2, k in enumerate(keys):
            if k not in run_start:
                run_start[k] = i2
        run_len = counts[c]
        out_pos = 0
        for g, segs in sched:
            for chp in range(N_CHUNKS):
                for wseg, nt in segs[chp]:
                    cnt = int(run_len[chp, wseg])
                    if cnt > 0:
                        i0 = run_start[(g, chp, wseg)]
                        sl = slice(i0, i0 + cnt)
                        local = (es[sl] - chp * CHUNK).astype(np.int16)
                        idx16[c, out_pos : out_pos + cnt] = local
                        slots[c, out_pos : out_pos + cnt] = eslot[sl].astype(
                            np.float32
                        )
                        uvals[c, out_pos : out_pos + cnt] = ue[sl].astype(np.float32)
                    out_pos += nt * 128
        assert out_pos == T * 128

    return u, n_tiles, sched, T, idx16, slots, uvals, perm


def _raw_gather(gp, mybir, out_ap, in_ap, idxs_ap, num_idxs, elem_size, elem_step):
    """dma_gather (non-transpose, HBM source) without the 256B-multiple
    elem restriction: elem_size may be any size as long as the source row
    STRIDE (elem_step) is a 256B multiple. Mirrors bass.BassGpSimd.dma_gather."""
    import concourse.ap_utils as ap_utils

    assert idxs_ap.dtype == mybir.dt.int16
    assert in_ap.dtype == out_ap.dtype
    stride_bytes = elem_step * mybir.dt.size(in_ap.dtype)
    assert stride_bytes % 256 == 0 and stride_bytes // 256 < 256
    assert ap_utils.ap_is_contiguous(in_ap.ap[1:])
    assert ap_utils.ap_is_contiguous(out_ap.ap[1:])
    assert ap_utils.ap_is_contiguous(idxs_ap.ap[1:])
    assert in_ap.ap[-1][1] == out_ap.ap[-1][1] == elem_size
    assert in_ap.ap[0][0] == elem_step
    _in_ap = gp.lower_ap_dma(in_ap, for_custom_bir_dma=True)
    _idxs_ap = gp.lower_ap(idxs_ap)
    _out_ap = gp.lower_ap(out_ap)
    return gp.add_instruction(
        mybir.InstDMAGatherAnt(
            name=gp.bass.get_next_instruction_name(),
            ins=[*_in_ap, _idxs_ap, gp.lower_val_access(gp.to_reg(num_idxs))],
            outs=[_out_ap],
            transpose=False,
            num_idxs=num_idxs,
            elem_size=elem_size,
            stride_bytes_256=stride_bytes // 256,
            gen_mode=0,
            single_packet=False,
            queue_num=0,
            sbuf_tokens_per_rank=0,
            sbuf_free_dim_per_rank=0,
            sbuf_free_dim_pad_per_rank=0,
            sbuf_byte_offset=0,
        )
    )


def _build_program(T, sched, has_bias):
    import concourse.bacc as bacc
    import concourse.mybir as mybir
    from concourse import tile

    f32 = mybir.dt.float32
    bf16 = mybir.dt.bfloat16
    fp8 = mybir.dt.float8e4

    nc = bacc.Bacc(
        "TRN2",
        target_bir_lowering=False,
        debug=False,
        enable_asserts=True,
        num_devices=N_CORES,
    )

    y_d = nc.dram_tensor("y8", [NPAD, YSTRIDE], fp8, kind="ExternalInput").ap()
    idx_d = nc.dram_tensor("idx16", [128, T * 8], mybir.dt.int16, kind="ExternalInput").ap()
    slots_d = nc.dram_tensor("slots", [128, T], f32, kind="ExternalInput").ap()
    uvals_d = nc.dram_tensor("uvals", [128, T], f32, kind="ExternalInput").ap()
    iota_d = nc.dram_tensor("iota", [128, 128], bf16, kind="ExternalInput").ap()
    ident_d = nc.dram_tensor("ident", [128, 128], bf16, kind="ExternalInput").ap()
    pidx_d = nc.dram_tensor("pidx", [128, 1], f32, kind="ExternalInput").ap()
    u2w_d = nc.dram_tensor("u2w", [128, WN], f32, kind="ExternalInput").ap()
    xsnm_d = nc.dram_tensor("xsnm", [128, WN * F], bf16, kind="ExternalInput").ap()
    # with a nonzero bias the self-loop seed needs plain x (not x+b)
    xnm_d = (nc.dram_tensor("xnm", [128, WN * F], bf16, kind="ExternalInput").ap()
             if has_bias else xsnm_d)
    w_d = nc.dram_tensor("W", [F, F], f32, kind="ExternalInput").ap()
    out_d = nc.dram_tensor("outT", [128, S], bf16, kind="ExternalOutput").ap()

    # last chunk with tiles, per window (for matmul stop flags); -1 = none
    last_ch = {}
    for g, segs in sched:
        for ch in range(N_CHUNKS):
            for w, nt in segs[ch]:
                last_ch[w] = ch

    with tile.TileContext(nc) as tc:
        with (
            tc.tile_pool(name="const", bufs=1) as const_p,
            tc.tile_pool(name="acc", bufs=1) as acc_p,
            tc.tile_pool(name="msgs", bufs=6) as msgs_p,
            tc.tile_pool(name="oh", bufs=4) as oh_p,
            tc.tile_pool(name="diag", bufs=4) as diag_p,
            tc.tile_pool(name="psum", bufs=8, space="PSUM") as psum_p,
            tc.tile_pool(name="fin", bufs=3) as fin_p,
        ):
            idx_sb = const_p.tile([128, T * 8], mybir.dt.int16)
            slots_sb = const_p.tile([128, T], f32)
            uvals_sb = const_p.tile([128, T], f32)
            iota_sb = const_p.tile([128, 128], bf16)
            ident_sb = const_p.tile([128, 128], bf16)
            pidx_sb = const_p.tile([128, 1], f32)
            u2w_sb = const_p.tile([128, WN], f32)
            xsnm_sb = const_p.tile([128, WN * F], bf16)
            xnm_sb = (const_p.tile([128, WN * F], bf16) if has_bias else xsnm_sb)
            w_sb = const_p.tile([F, F], f32)
            acc = acc_p.tile([128, S], f32)

            # consts needed by group-0 compute load first; W (tail-only)
            # is deferred into the loop so it doesn't delay the first gathers
            nc.sync.dma_start(idx_sb[:], idx_d[:])
            nc.sync.dma_start(slots_sb[:], slots_d[:])
            nc.sync.dma_start(uvals_sb[:], uvals_d[:])
            nc.sync.dma_start(iota_sb[:], iota_d[:])
            nc.sync.dma_start(ident_sb[:], ident_d[:])
            nc.sync.dma_start(pidx_sb[:], pidx_d[:])
            nc.sync.dma_start(u2w_sb[:], u2w_d[:])
            nc.sync.dma_start(xsnm_sb[:], xsnm_d[:])
            if has_bias:
                nc.sync.dma_start(xnm_sb[:], xnm_d[:])

            SL = 512

            def emit_tail(s0, n):
                # out^T chunk = W^T @ acc + xs (xs seeded via identity)
                pf = psum_p.tile([128, SL], f32, tag="psum", name=f"pf_{s0}")
                for j in range((n + 127) // 128):
                    w = s0 // 128 + j
                    nc.tensor.matmul(
                        pf[:, j * 128 : j * 128 + 128],
                        lhsT=xsnm_sb[:, w * F : (w + 1) * F],
                        rhs=ident_sb[:],
                        start=(j == 0),
                        stop=False,
                    )
                nc.tensor.matmul(pf[:, :n], lhsT=w_sb[:], rhs=acc[:, s0 : s0 + n],
                                 start=False, stop=True)
                ot = fin_p.tile([128, SL], bf16, tag="ot")
                nc.scalar.copy(out=ot[:, :n], in_=pf[:, :n])
                nc.sync.dma_start(out_d[:, s0 : s0 + n], ot[:, :n])

            g_tile = 0   # global tile cursor
            tail_s0 = 0  # next output chunk to emit
            for g, segs in sched:
                ws = list(range(g * WG, min((g + 1) * WG, WN)))
                nbank = (len(ws) + 3) // 4
                banks = [psum_p.tile([128, 512], f32, tag="psum", name=f"ps_g{g}_{b}")
                         for b in range(nbank)]

                def wslice(w):
                    wl = w - g * WG
                    return banks[wl // 4][:, (wl % 4) * 128 : (wl % 4) * 128 + 128]

                # seed each window's PSUM quarter with the self-loop term
                # u^2*x via a diagonal rhs: diag[p, j] = (j == p) * u^2_p.
                # start=True zeroes the whole 2KB bank, so only the FIRST
                # matmul touching a bank may set it; later seeds add onto
                # the zeroed bank with start=False.
                for w in ws:
                    wl = w - g * WG
                    dg = diag_p.tile([128, 128], bf16, tag="diag")
                    nc.gpsimd.tensor_scalar(
                        dg[:],
                        iota_sb[:],
                        pidx_sb[:, 0:1],
                        u2w_sb[:, w : w + 1],
                        mybir.AluOpType.is_equal,
                        mybir.AluOpType.mult,
                    )
                    nc.tensor.matmul(
                        wslice(w),
                        lhsT=xnm_sb[:, w * F : (w + 1) * F],
                        rhs=dg[:],
                        start=(wl % 4 == 0),
                        stop=w not in last_ch,
                    )
                for ch in range(N_CHUNKS):
                    seg_tiles = sum(nt for (_, nt) in segs[ch])
                    if seg_tiles == 0:
                        continue
                    flat = [(wseg, k, nt) for wseg, nt in segs[ch]
                            for k in range(nt)]
                    # split into bounded sub-gathers so msgs buffers stay
                    # small enough to multi-buffer across groups
                    GMAX = 64
                    for sub0 in range(0, seg_tiles, GMAX):
                        sub = flat[sub0 : sub0 + GMAX]
                        ntile = len(sub)
                        n_idx = ntile * 128
                        base = g_tile + sub0
                        msgs = msgs_p.tile([128, ntile * 128], fp8, tag="msgs")
                        m3 = msgs[:].rearrange("p (b f) -> p b f", f=F)
                        _raw_gather(
                            nc.gpsimd, mybir, m3,
                            y_d[ch * CHUNK : (ch + 1) * CHUNK, 0:F],
                            idx_sb[:, base * 8 : base * 8 + n_idx // 16],
                            n_idx, F, YSTRIDE,
                        )
                        # one-hots in 8-tile super-tiles so the tile framework
                        # batches buffer-reuse waits (PE consumes in-order)
                        OHB = 8
                        oh_sup = None
                        for ti, (wseg, k, nt) in enumerate(sub):
                            ob = ti % OHB
                            if ob == 0:
                                oh_sup = oh_p.tile([128, OHB * 128], bf16)
                            gt = base + ti
                            # oh[e, j] = (iota_j == slot_e) * u[dst_e]
                            nc.vector.tensor_scalar(
                                oh_sup[:, ob * 128 : ob * 128 + 128],
                                iota_sb[:],
                                slots_sb[:, gt : gt + 1],
                                uvals_sb[:, gt : gt + 1],
                                mybir.AluOpType.is_equal,
                                mybir.AluOpType.mult,
                            )
                            nc.tensor.matmul(
                                wslice(wseg),
                                lhsT=msgs[:, ti * 128 : (ti + 1) * 128],
                                rhs=oh_sup[:, ob * 128 : ob * 128 + 128],
                                start=False,
                                stop=(last_ch[wseg] == ch and k == nt - 1),
                            )
                    g_tile += seg_tiles
                if g == 0:
                    # tail-only const: queued behind group 0's gathers so it
                    # doesn't delay the pipeline start, but emitted before
                    # the first tail chunk reads it
                    nc.sync.dma_start(w_sb[:], w_d[:])
                # flush: Act copies PSUM banks into the (write-once) acc
                for b in range(nbank):
                    c0 = (g * WG + b * 4) * 128
                    ncols = min(512, S - c0)
                    nc.scalar.copy(out=acc[:, c0 : c0 + ncols], in_=banks[b][:, :ncols])
                # emit output chunks whose acc columns are fully flushed
                flushed = min((g + 1) * WG, WN) * 128
                while tail_s0 < S and tail_s0 + min(SL, S - tail_s0) <= flushed:
                    n = min(SL, S - tail_s0)
                    emit_tail(tail_s0, n)
                    tail_s0 += n
            assert g_tile == T
            assert tail_s0 == S

    nc.compile()
    return nc


_PROGRAM_CACHE = {}


def _get_program(T, sched, has_bias):
    key = (T, has_bias,
           tuple((g, tuple(tuple(seg) for seg in segs)) for g, segs in sched))
    if key not in _PROGRAM_CACHE:
        _PROGRAM_CACHE[key] = _build_program(T, sched, has_bias)
    return _PROGRAM_CACHE[key]


def _prepare(x, edge_index, W, b):
    x = np.asarray(x, dtype=np.float32)
    edge_index = np.asarray(edge_index)
    W = np.asarray(W, dtype=np.float32)
    b = np.asarray(b, dtype=np.float32)

    u, n_tiles, sched, T, idx16, slots, uvals, perm = _host_plan(edge_index)

    import ml_dtypes
    bf = ml_dtypes.bfloat16
    f8 = ml_dtypes.float8_e4m3
    y8 = np.zeros((NPAD, YSTRIDE), dtype=f8)
    y8[:N_NODES, :F] = (u[:, None] * x).astype(f8)

    iota = np.tile(np.arange(128, dtype=np.float32), (128, 1)).astype(bf)
    ident = np.eye(128, dtype=np.float32).astype(bf)

    u_ext = np.concatenate([u, [0.0]]).astype(np.float32)
    x_ext = np.concatenate([x, np.zeros((1, F), np.float32)], axis=0)
    xs_ext = x_ext + b[None, :]
    has_bias = bool(np.any(b != 0))
    pidx = np.arange(128, dtype=np.float32).reshape(128, 1)

    in_maps = []
    for c in range(N_CORES):
        rows = perm[c]
        idx_c = np.tile(idx16[c].reshape(-1, 16).T, (8, 1)).copy()  # [128, T*8]
        slots_c = slots[c].reshape(T, 128).T.copy()
        uvals_c = uvals[c].reshape(T, 128).T.copy()
        # node-major per-window tiles: [slot-partition, window, feature]
        xsnm = xs_ext[rows].astype(bf).reshape(WN, 128, F).transpose(1, 0, 2)
        u2w = (u_ext[rows] ** 2).astype(np.float32).reshape(WN, 128).T
        im = {
            "y8": y8,
            "idx16": idx_c,
            "slots": slots_c.astype(np.float32),
            "uvals": uvals_c.astype(np.float32),
            "iota": iota,
            "ident": ident,
            "pidx": pidx,
            "u2w": np.ascontiguousarray(u2w),
            "xsnm": np.ascontiguousarray(xsnm.reshape(128, WN * F)),
            "W": W,
        }
        if has_bias:
            xnm = x_ext[rows].astype(bf).reshape(WN, 128, F).transpose(1, 0, 2)
            im["xnm"] = np.ascontiguousarray(xnm.reshape(128, WN * F))
        in_maps.append(im)

    nc = _get_program(T, sched, has_bias)
    global _LAST_PERM
    _LAST_PERM = perm
    return nc, in_maps


_LAST_PERM = None


def _unshard(results, perm=None):
    if perm is None:
        perm = _LAST_PERM
    out = np.empty((N_NODES, F), dtype=np.float32)
    for c in range(N_CORES):
        rows = perm[c]
        valid = rows >= 0
        out[rows[valid]] = results[c]["outT"].T.astype(np.float32)[valid]
    return out


def kernel(x, edge_index, W, b):
    from concourse.bass_utils import run_bass_kernel_spmd

    nc, in_maps = _prepare(x, edge_index, W, b)
    res = run_bass_kernel_spmd(nc, in_maps, list(range(N_CORES)))
    return _unshard(res.results)


if __name__ == "__main__":
    rng = np.random.default_rng(0)
    x = rng.standard_normal((N_NODES, F), dtype=np.float32)
    ei = rng.integers(0, N_NODES, size=(2, 1600000)).astype(np.int64)
    W = rng.standard_normal((F, F), dtype=np.float32) / np.sqrt(F)
    b = np.zeros(F, dtype=np.float32)
    out = kernel(x=x, edge_index=ei, W=W, b=b)
    print(out.shape, out.dtype)


# revision 27
# speedup vs baseline: 1.0111x; 1.0111x over previous
"""GCNConvSC (residual + GCNConv) Trainium2 Bass kernel, 8-core SPMD.

Math (matches the PyG-style reference):
    deg[v]  = indeg_with_selfloop(v)          (count of v in dst, +1)
    u       = deg^{-1/2}
    y       = u[:,None] * x                   (pre-scaled node features, fp8)
    z[v]    = sum_{e: dst_e = v} y[src_e] * u[v]   (via one-hot matmuls)
    out[v]  = x[v] + b + (z[v] + u[v]^2 * x[v]) @ W

Pipeline per core (dst nodes range-partitioned, S=12544 slots, 98 windows
of 128):
  - y stored in HBM as fp8 e4m3 rows padded to a 256B stride; per-edge rows
    are fetched with a raw InstDMAGatherAnt (elem_size=128, elem_step=256),
    i.e. 128B descriptors, which the DMA cost model prices at half the
    256B-descriptor rate.  Edges are bucketed by (window-group, src-chunk,
    window) with int16 chunk-local indices (4 chunks of 25024 rows).
  - Aggregation: per 128-edge tile a bf16 one-hot (iota==slot)*u[dst] is
    built on DVE (4x perf mode) and matmul'd (fp8 lhsT x bf16 rhs) into a
    PSUM bank quarter for the edge's dst window.
  - The self-loop term ys = u^2*x and the residual xs = x + b are seeded
    into PSUM by identity-rhs matmuls (lhsT = node-major bf16 tiles), so
    the SBUF accumulator is write-once and flushes are plain Activation-
    engine PSUM->SBUF copies (DVE stays free for one-hots).
  - Tail: out^T = W^T @ acc accumulated on top of the xs seed, copied to
    bf16 and stored.
"""

import sys

sys.path.insert(0, "/opt/trn_rl_repo")

import numpy as np

N_NODES = 100000
F = 128
N_CORES = 8
S = 12544            # dst slots per core (98 windows of 128)
WN = 98              # windows per core
WG = 16              # windows per PSUM group (4 banks of 4 windows)
N_CHUNKS = 4
CHUNK = 25024        # gather-source rows per chunk (int16-safe)
NPAD = N_CHUNKS * CHUNK  # 100096 padded node rows for y
YSTRIDE = 256        # fp8 row stride in bytes (DMA desc stride granularity)


def _host_plan(edge_index):
    """Sort/bucket edges per core; emit the shared SPMD schedule plus
    per-core gather-index and slot arrays."""
    src = np.asarray(edge_index[0], dtype=np.int64)
    dst = np.asarray(edge_index[1], dtype=np.int64)

    deg_e = np.bincount(dst, minlength=N_NODES)
    u = (1.0 / np.sqrt(deg_e.astype(np.float64) + 1.0)).astype(np.float32)

    chunk_of = src // CHUNK

    # Window-classes of 1024 similar-degree dsts (descending degree); within
    # each class, greedily deal the dsts to the 8 cores balancing the
    # per-chunk edge-count vectors, so the shared max-over-cores schedule
    # pads as little as possible.
    dvec = np.zeros((N_NODES, N_CHUNKS), np.int64)
    np.add.at(dvec, (dst, chunk_of), 1)
    order = np.argsort(-deg_e, kind="stable")

    perm = np.full((N_CORES, S), -1, dtype=np.int64)
    core_of_node = np.empty(N_NODES, dtype=np.int64)
    pos_of_node = np.empty(N_NODES, dtype=np.int64)
    inf = np.float64(1e18)
    for w in range(WN):
        cls = order[w * 128 * N_CORES : (w + 1) * 128 * N_CORES]
        L = np.zeros((N_CORES, N_CHUNKS), np.float64)
        cap = np.full(N_CORES, 128, np.int64)
        nxt = np.full(N_CORES, w * 128, np.int64)
        for v in cls:
            cost = ((L + dvec[v]) ** 2).sum(axis=1)
            cost[cap == 0] = inf
            c = int(np.argmin(cost))
            L[c] += dvec[v]
            cap[c] -= 1
            perm[c, nxt[c]] = v
            core_of_node[v] = c
            pos_of_node[v] = nxt[c]
            nxt[c] += 1

    core_of = core_of_node[dst]
    pos_e_all = pos_of_node[dst]
    u_e_all = u[dst]

    per_core = []
    counts = np.zeros((N_CORES, N_CHUNKS, WN), dtype=np.int64)
    for c in range(N_CORES):
        m = core_of == c
        es, pos_e, ue = src[m], pos_e_all[m], u_e_all[m]
        ch = chunk_of[m]
        w = pos_e // 128
        slot = pos_e % 128
        wg = w // WG
        so = np.lexsort((w, ch, wg))
        es, slot, ch, w, ue = es[so], slot[so], ch[so], w[so], ue[so]
        np.add.at(counts[c], (ch, w), 1)
        per_core.append((es, slot, ch, w, ue))

    # shared schedule: tiles per (chunk, window) = max over cores; windows
    # with zero edges need no tiles (their PSUM quarter is seeded anyway)
    n_tiles = (counts.max(axis=0) + 127) // 128  # [N_CHUNKS, WN]

    n_wg = (WN + WG - 1) // WG
    sched = []  # (g, segs) with segs[ch] = [(window, ntiles), ...]
    T = 0
    for g in range(n_wg):
        ws = range(g * WG, min((g + 1) * WG, WN))
        segs = []
        for ch in range(N_CHUNKS):
            tl = [(w, int(n_tiles[ch, w])) for w in ws if n_tiles[ch, w] > 0]
            segs.append(tl)
        sched.append((g, segs))
        T += int(n_tiles[:, list(ws)].sum())

    # per-core padded edge streams in schedule order
    idx16 = np.zeros((N_CORES, T * 128), dtype=np.int16)
    slots = np.full((N_CORES, T * 128), -1.0, dtype=np.float32)
    uvals = np.zeros((N_CORES, T * 128), dtype=np.float32)
    for c in range(N_CORES):
        es, eslot, ch, w, ue = per_core[c]
        keys = list(zip(w // WG, ch, w))
        run_start = {}
        for i2, k in enumerate(keys):
            if k not in run_start:
                run_start[k] = i2
        run_len = counts[c]
        out_pos = 0
        for g, segs in sched:
            for chp in range(N_CHUNKS):
                for wseg, nt in segs[chp]:
                    cnt = int(run_len[chp, wseg])
                    if cnt > 0:
                        i0 = run_start[(g, chp, wseg)]
                        sl = slice(i0, i0 + cnt)
                        local = (es[sl] - chp * CHUNK).astype(np.int16)
                        idx16[c, out_pos : out_pos + cnt] = local
                        slots[c, out_pos : out_pos + cnt] = eslot[sl].astype(
                            np.float32
                        )
                        uvals[c, out_pos : out_pos + cnt] = ue[sl].astype(np.float32)
                    out_pos += nt * 128
        assert out_pos == T * 128

    return u, n_tiles, sched, T, idx16, slots, uvals, perm


def _raw_gather(gp, mybir, out_ap, in_ap, idxs_ap, num_idxs, elem_size, elem_step):
    """dma_gather (non-transpose, HBM source) without the 256B-multiple
    elem restriction: elem_size may be any size as long as the source row
    STRIDE (elem_step) is a 256B multiple. Mirrors bass.BassGpSimd.dma_gather."""
    import concourse.ap_utils as ap_utils

    assert idxs_ap.dtype == mybir.dt.int16
    assert in_ap.dtype == out_ap.dtype
    stride_bytes = elem_step * mybir.dt.size(in_ap.dtype)
    assert stride_bytes % 256 == 0 and stride_bytes // 256 < 256
    assert ap_utils.ap_is_contiguous(in_ap.ap[1:])
    assert ap_utils.ap_is_contiguous(out_ap.ap[1:])
    assert ap_utils.ap_is_contiguous(idxs_ap.ap[1:])
    assert in_ap.ap[-1][1] == out_ap.ap[-1][1] == elem_size
    assert in_ap.ap[0][0] == elem_step
    _in_ap = gp.lower_ap_dma(in_ap, for_custom_bir_dma=True)
    _idxs_ap = gp.lower_ap(idxs_ap)
    _out_ap = gp.lower_ap(out_ap)
    return gp.add_instruction(
        mybir.InstDMAGatherAnt(
            name=gp.bass.get_next_instruction_name(),
            ins=[*_in_ap, _idxs_ap, gp.lower_val_access(gp.to_reg(num_idxs))],
            outs=[_out_ap],
            transpose=False,
            num_idxs=num_idxs,
            elem_size=elem_size,
            stride_bytes_256=stride_bytes // 256,
            gen_mode=0,
            single_packet=False,
            queue_num=0,
            sbuf_tokens_per_rank=0,
            sbuf_free_dim_per_rank=0,
            sbuf_free_dim_pad_per_rank=0,
            sbuf_byte_offset=0,
        )
    )


def _build_program(T, sched, has_bias):
    import concourse.bacc as bacc
    import concourse.mybir as mybir
    from concourse import tile

    f32 = mybir.dt.float32
    bf16 = mybir.dt.bfloat16
    fp8 = mybir.dt.float8e4

    nc = bacc.Bacc(
        "TRN2",
        target_bir_lowering=False,
        debug=False,
        enable_asserts=True,
        num_devices=N_CORES,
    )

    y_d = nc.dram_tensor("y8", [NPAD, YSTRIDE], fp8, kind="ExternalInput").ap()
    idx_d = nc.dram_tensor("idx16", [128, T * 8], mybir.dt.int16, kind="ExternalInput").ap()
    slots_d = nc.dram_tensor("slots", [128, T], f32, kind="ExternalInput").ap()
    uvals_d = nc.dram_tensor("uvals", [128, T], f32, kind="ExternalInput").ap()
    iota_d = nc.dram_tensor("iota", [128, 128], bf16, kind="ExternalInput").ap()
    ident_d = nc.dram_tensor("ident", [128, 128], bf16, kind="ExternalInput").ap()
    pidx_d = nc.dram_tensor("pidx", [128, 1], f32, kind="ExternalInput").ap()
    u2w_d = nc.dram_tensor("u2w", [128, WN], f32, kind="ExternalInput").ap()
    xsnm_d = nc.dram_tensor("xsnm", [128, WN * F], bf16, kind="ExternalInput").ap()
    # with a nonzero bias the self-loop seed needs plain x (not x+b)
    xnm_d = (nc.dram_tensor("xnm", [128, WN * F], bf16, kind="ExternalInput").ap()
             if has_bias else xsnm_d)
    w_d = nc.dram_tensor("W", [F, F], f32, kind="ExternalInput").ap()
    out_d = nc.dram_tensor("outT", [128, S], bf16, kind="ExternalOutput").ap()

    # last chunk with tiles, per window (for matmul stop flags); -1 = none
    last_ch = {}
    for g, segs in sched:
        for ch in range(N_CHUNKS):
            for w, nt in segs[ch]:
                last_ch[w] = ch

    with tile.TileContext(nc) as tc:
        with (
            tc.tile_pool(name="const", bufs=1) as const_p,
            tc.tile_pool(name="acc", bufs=1) as acc_p,
            tc.tile_pool(name="msgs", bufs=6) as msgs_p,
            tc.tile_pool(name="oh", bufs=4) as oh_p,
            tc.tile_pool(name="diag", bufs=4) as diag_p,
            tc.tile_pool(name="psum", bufs=8, space="PSUM") as psum_p,
            tc.tile_pool(name="fin", bufs=3) as fin_p,
        ):
            idx_sb = const_p.tile([128, T * 8], mybir.dt.int16)
            slots_sb = const_p.tile([128, T], f32)
            uvals_sb = const_p.tile([128, T], f32)
            iota_sb = const_p.tile([128, 128], bf16)
            ident_sb = const_p.tile([128, 128], bf16)
            pidx_sb = const_p.tile([128, 1], f32)
            u2w_sb = const_p.tile([128, WN], f32)
            xsnm_sb = const_p.tile([128, WN * F], bf16)
            xnm_sb = (const_p.tile([128, WN * F], bf16) if has_bias else xsnm_sb)
            w_sb = const_p.tile([F, F], f32)
            acc = acc_p.tile([128, S], f32)

            # consts needed by group-0 compute load first; W (tail-only)
            # is deferred into the loop so it doesn't delay the first gathers
            nc.sync.dma_start(idx_sb[:], idx_d[:])
            nc.sync.dma_start(slots_sb[:], slots_d[:])
            nc.sync.dma_start(uvals_sb[:], uvals_d[:])
            nc.sync.dma_start(iota_sb[:], iota_d[:])
            nc.sync.dma_start(ident_sb[:], ident_d[:])
            nc.sync.dma_start(pidx_sb[:], pidx_d[:])
            nc.sync.dma_start(u2w_sb[:], u2w_d[:])
            nc.sync.dma_start(xsnm_sb[:], xsnm_d[:])
            if has_bias:
                nc.sync.dma_start(xnm_sb[:], xnm_d[:])

            SL = 512

            def emit_tail(s0, n):
                # out^T chunk = W^T @ acc + xs (xs seeded via identity)
                pf = psum_p.tile([128, SL], f32, tag="psum", name=f"pf_{s0}")
                for j in range((n + 127) // 128):
                    w = s0 // 128 + j
                    nc.tensor.matmul(
                        pf[:, j * 128 : j * 128 + 128],
                        lhsT=xsnm_sb[:, w * F : (w + 1) * F],
                        rhs=ident_sb[:],
                        start=(j == 0),
                        stop=False,
                    )
                nc.tensor.matmul(pf[:, :n], lhsT=w_sb[:], rhs=acc[:, s0 : s0 + n],
                                 start=False, stop=True)
                ot = fin_p.tile([128, SL], bf16, tag="ot")
                nc.scalar.copy(out=ot[:, :n], in_=pf[:, :n])
                nc.sync.dma_start(out_d[:, s0 : s0 + n], ot[:, :n])

            g_tile = 0   # global tile cursor
            tail_s0 = 0  # next output chunk to emit
            for g, segs in sched:
                ws = list(range(g * WG, min((g + 1) * WG, WN)))
                nbank = (len(ws) + 3) // 4
                banks = [psum_p.tile([128, 512], f32, tag="psum", name=f"ps_g{g}_{b}")
                         for b in range(nbank)]

                def wslice(w):
                    wl = w - g * WG
                    return banks[wl // 4][:, (wl % 4) * 128 : (wl % 4) * 128 + 128]

                # seed each window's PSUM quarter with the self-loop term
                # u^2*x via a diagonal rhs: diag[p, j] = (j == p) * u^2_p.
                # start=True zeroes the whole 2KB bank, so only the FIRST
                # matmul touching a bank may set it; later seeds add onto
                # the zeroed bank with start=False.
                for w in ws:
                    wl = w - g * WG
                    dg = diag_p.tile([128, 128], bf16, tag="diag")
                    nc.vector.tensor_scalar(
                        dg[:],
                        iota_sb[:],
                        pidx_sb[:, 0:1],
                        u2w_sb[:, w : w + 1],
                        mybir.AluOpType.is_equal,
                        mybir.AluOpType.mult,
                    )
                    nc.tensor.matmul(
                        wslice(w),
                        lhsT=xnm_sb[:, w * F : (w + 1) * F],
                        rhs=dg[:],
                        start=(wl % 4 == 0),
                        stop=w not in last_ch,
                    )
                for ch in range(N_CHUNKS):
                    seg_tiles = sum(nt for (_, nt) in segs[ch])
                    if seg_tiles == 0:
                        continue
                    flat = [(wseg, k, nt) for wseg, nt in segs[ch]
                            for k in range(nt)]
                    # split into bounded sub-gathers so msgs buffers stay
                    # small enough to multi-buffer across groups
                    GMAX = 48
                    for sub0 in range(0, seg_tiles, GMAX):
                        sub = flat[sub0 : sub0 + GMAX]
                        ntile = len(sub)
                        n_idx = ntile * 128
                        base = g_tile + sub0
                        msgs = msgs_p.tile([128, ntile * 128], fp8, tag="msgs")
                        m3 = msgs[:].rearrange("p (b f) -> p b f", f=F)
                        _raw_gather(
                            nc.gpsimd, mybir, m3,
                            y_d[ch * CHUNK : (ch + 1) * CHUNK, 0:F],
                            idx_sb[:, base * 8 : base * 8 + n_idx // 16],
                            n_idx, F, YSTRIDE,
                        )
                        # one-hots in 8-tile super-tiles so the tile framework
                        # batches buffer-reuse waits (PE consumes in-order)
                        OHB = 8
                        oh_sup = None
                        for ti, (wseg, k, nt) in enumerate(sub):
                            ob = ti % OHB
                            if ob == 0:
                                oh_sup = oh_p.tile([128, OHB * 128], bf16)
                            gt = base + ti
                            # oh[e, j] = (iota_j == slot_e) * u[dst_e]
                            nc.vector.tensor_scalar(
                                oh_sup[:, ob * 128 : ob * 128 + 128],
                                iota_sb[:],
                                slots_sb[:, gt : gt + 1],
                                uvals_sb[:, gt : gt + 1],
                                mybir.AluOpType.is_equal,
                                mybir.AluOpType.mult,
                            )
                            nc.tensor.matmul(
                                wslice(wseg),
                                lhsT=msgs[:, ti * 128 : (ti + 1) * 128],
                                rhs=oh_sup[:, ob * 128 : ob * 128 + 128],
                                start=False,
                                stop=(last_ch[wseg] == ch and k == nt - 1),
                            )
                    g_tile += seg_tiles
                if g == 0:
                    # tail-only const: queued behind group 0's gathers so it
                    # doesn't delay the pipeline start, but emitted before
                    # the first tail chunk reads it
                    nc.sync.dma_start(w_sb[:], w_d[:])
                # flush: Act copies PSUM banks into the (write-once) acc
                for b in range(nbank):
                    c0 = (g * WG + b * 4) * 128
                    ncols = min(512, S - c0)
                    nc.scalar.copy(out=acc[:, c0 : c0 + ncols], in_=banks[b][:, :ncols])
                # emit output chunks whose acc columns are fully flushed
                flushed = min((g + 1) * WG, WN) * 128
                while tail_s0 < S and tail_s0 + min(SL, S - tail_s0) <= flushed:
                    n = min(SL, S - tail_s0)
                    emit_tail(tail_s0, n)
                    tail_s0 += n
            assert g_tile == T
            assert tail_s0 == S

    nc.compile()
    return nc


_PROGRAM_CACHE = {}


def _get_program(T, sched, has_bias):
    key = (T, has_bias,
           tuple((g, tuple(tuple(seg) for seg in segs)) for g, segs in sched))
    if key not in _PROGRAM_CACHE:
        _PROGRAM_CACHE[key] = _build_program(T, sched, has_bias)
    return _PROGRAM_CACHE[key]


def _prepare(x, edge_index, W, b):
    x = np.asarray(x, dtype=np.float32)
    edge_index = np.asarray(edge_index)
    W = np.asarray(W, dtype=np.float32)
    b = np.asarray(b, dtype=np.float32)

    u, n_tiles, sched, T, idx16, slots, uvals, perm = _host_plan(edge_index)

    import ml_dtypes
    bf = ml_dtypes.bfloat16
    f8 = ml_dtypes.float8_e4m3
    y8 = np.zeros((NPAD, YSTRIDE), dtype=f8)
    y8[:N_NODES, :F] = (u[:, None] * x).astype(f8)

    iota = np.tile(np.arange(128, dtype=np.float32), (128, 1)).astype(bf)
    ident = np.eye(128, dtype=np.float32).astype(bf)

    u_ext = np.concatenate([u, [0.0]]).astype(np.float32)
    x_ext = np.concatenate([x, np.zeros((1, F), np.float32)], axis=0)
    xs_ext = x_ext + b[None, :]
    has_bias = bool(np.any(b != 0))
    pidx = np.arange(128, dtype=np.float32).reshape(128, 1)

    in_maps = []
    for c in range(N_CORES):
        rows = perm[c]
        idx_c = np.tile(idx16[c].reshape(-1, 16).T, (8, 1)).copy()  # [128, T*8]
        slots_c = slots[c].reshape(T, 128).T.copy()
        uvals_c = uvals[c].reshape(T, 128).T.copy()
        # node-major per-window tiles: [slot-partition, window, feature]
        xsnm = xs_ext[rows].astype(bf).reshape(WN, 128, F).transpose(1, 0, 2)
        u2w = (u_ext[rows] ** 2).astype(np.float32).reshape(WN, 128).T
        im = {
            "y8": y8,
            "idx16": idx_c,
            "slots": slots_c.astype(np.float32),
            "uvals": uvals_c.astype(np.float32),
            "iota": iota,
            "ident": ident,
            "pidx": pidx,
            "u2w": np.ascontiguousarray(u2w),
            "xsnm": np.ascontiguousarray(xsnm.reshape(128, WN * F)),
            "W": W,
        }
        if has_bias:
            xnm = x_ext[rows].astype(bf).reshape(WN, 128, F).transpose(1, 0, 2)
            im["xnm"] = np.ascontiguousarray(xnm.reshape(128, WN * F))
        in_maps.append(im)

    nc = _get_program(T, sched, has_bias)
    global _LAST_PERM
    _LAST_PERM = perm
    return nc, in_maps


_LAST_PERM = None


def _unshard(results, perm=None):
    if perm is None:
        perm = _LAST_PERM
    out = np.empty((N_NODES, F), dtype=np.float32)
    for c in range(N_CORES):
        rows = perm[c]
        valid = rows >= 0
        out[rows[valid]] = results[c]["outT"].T.astype(np.float32)[valid]
    return out


def kernel(x, edge_index, W, b):
    from concourse.bass_utils import run_bass_kernel_spmd

    nc, in_maps = _prepare(x, edge_index, W, b)
    res = run_bass_kernel_spmd(nc, in_maps, list(range(N_CORES)))
    return _unshard(res.results)


if __name__ == "__main__":
    rng = np.random.default_rng(0)
    x = rng.standard_normal((N_NODES, F), dtype=np.float32)
    ei = rng.integers(0, N_NODES, size=(2, 1600000)).astype(np.int64)
    W = rng.standard_normal((F, F), dtype=np.float32) / np.sqrt(F)
    b = np.zeros(F, dtype=np.float32)
    out = kernel(x=x, edge_index=ei, W=W, b=b)
    print(out.shape, out.dtype)


# revision 28
# speedup vs baseline: 1.0467x; 1.0352x over previous
"""GCNConvSC (residual + GCNConv) Trainium2 Bass kernel, 8-core SPMD.

Math (matches the PyG-style reference):
    deg[v]  = indeg_with_selfloop(v)          (count of v in dst, +1)
    u       = deg^{-1/2}
    y       = u[:,None] * x                   (pre-scaled node features, fp8)
    z[v]    = sum_{e: dst_e = v} y[src_e] * u[v]   (via one-hot matmuls)
    out[v]  = x[v] + b + (z[v] + u[v]^2 * x[v]) @ W

Pipeline per core (dst nodes range-partitioned, S=12544 slots, 98 windows
of 128):
  - y stored in HBM as fp8 e4m3 rows padded to a 256B stride; per-edge rows
    are fetched with a raw InstDMAGatherAnt (elem_size=128, elem_step=256),
    i.e. 128B descriptors, which the DMA cost model prices at half the
    256B-descriptor rate.  Edges are bucketed by (window-group, src-chunk,
    window) with int16 chunk-local indices (4 chunks of 25024 rows).
  - Aggregation: per 128-edge tile a bf16 one-hot (iota==slot)*u[dst] is
    built on DVE (4x perf mode) and matmul'd (fp8 lhsT x bf16 rhs) into a
    PSUM bank quarter for the edge's dst window.
  - The self-loop term ys = u^2*x and the residual xs = x + b are seeded
    into PSUM by identity-rhs matmuls (lhsT = node-major bf16 tiles), so
    the SBUF accumulator is write-once and flushes are plain Activation-
    engine PSUM->SBUF copies (DVE stays free for one-hots).
  - Tail: out^T = W^T @ acc accumulated on top of the xs seed, copied to
    bf16 and stored.
"""

import sys

sys.path.insert(0, "/opt/trn_rl_repo")

import numpy as np

N_NODES = 100000
F = 128
N_CORES = 8
S = 12544            # dst slots per core (98 windows of 128)
WN = 98              # windows per core
WG = 16              # windows per PSUM group (4 banks of 4 windows)
N_CHUNKS = 4
CHUNK = 25024        # gather-source rows per chunk (int16-safe)
NPAD = N_CHUNKS * CHUNK  # 100096 padded node rows for y
YSTRIDE = 256        # fp8 row stride in bytes (DMA desc stride granularity)


def _host_plan(edge_index):
    """Sort/bucket edges per core; emit the shared SPMD schedule plus
    per-core gather-index and slot arrays."""
    src = np.asarray(edge_index[0], dtype=np.int64)
    dst = np.asarray(edge_index[1], dtype=np.int64)

    deg_e = np.bincount(dst, minlength=N_NODES)
    u = (1.0 / np.sqrt(deg_e.astype(np.float64) + 1.0)).astype(np.float32)

    chunk_of = src // CHUNK

    # Window-classes of 1024 similar-degree dsts (descending degree); within
    # each class, greedily deal the dsts to the 8 cores balancing the
    # per-chunk edge-count vectors, so the shared max-over-cores schedule
    # pads as little as possible.
    dvec = np.zeros((N_NODES, N_CHUNKS), np.int64)
    np.add.at(dvec, (dst, chunk_of), 1)
    order = np.argsort(-deg_e, kind="stable")

    perm = np.full((N_CORES, S), -1, dtype=np.int64)
    core_of_node = np.empty(N_NODES, dtype=np.int64)
    pos_of_node = np.empty(N_NODES, dtype=np.int64)
    inf = np.float64(1e18)
    for w in range(WN):
        cls = order[w * 128 * N_CORES : (w + 1) * 128 * N_CORES]
        L = np.zeros((N_CORES, N_CHUNKS), np.float64)
        cap = np.full(N_CORES, 128, np.int64)
        nxt = np.full(N_CORES, w * 128, np.int64)
        for v in cls:
            cost = ((L + dvec[v]) ** 2).sum(axis=1)
            cost[cap == 0] = inf
            c = int(np.argmin(cost))
            L[c] += dvec[v]
            cap[c] -= 1
            perm[c, nxt[c]] = v
            core_of_node[v] = c
            pos_of_node[v] = nxt[c]
            nxt[c] += 1

    core_of = core_of_node[dst]
    pos_e_all = pos_of_node[dst]
    u_e_all = u[dst]

    per_core = []
    counts = np.zeros((N_CORES, N_CHUNKS, WN), dtype=np.int64)
    for c in range(N_CORES):
        m = core_of == c
        es, pos_e, ue = src[m], pos_e_all[m], u_e_all[m]
        ch = chunk_of[m]
        w = pos_e // 128
        slot = pos_e % 128
        wg = w // WG
        so = np.lexsort((w, ch, wg))
        es, slot, ch, w, ue = es[so], slot[so], ch[so], w[so], ue[so]
        np.add.at(counts[c], (ch, w), 1)
        per_core.append((es, slot, ch, w, ue))

    # shared schedule: tiles per (chunk, window) = max over cores; windows
    # with zero edges need no tiles (their PSUM quarter is seeded anyway)
    n_tiles = (counts.max(axis=0) + 127) // 128  # [N_CHUNKS, WN]

    n_wg = (WN + WG - 1) // WG
    sched = []  # (g, segs) with segs[ch] = [(window, ntiles), ...]
    T = 0
    for g in range(n_wg):
        ws = range(g * WG, min((g + 1) * WG, WN))
        segs = []
        for ch in range(N_CHUNKS):
            tl = [(w, int(n_tiles[ch, w])) for w in ws if n_tiles[ch, w] > 0]
            segs.append(tl)
        sched.append((g, segs))
        T += int(n_tiles[:, list(ws)].sum())

    # per-core padded edge streams in schedule order
    idx16 = np.zeros((N_CORES, T * 128), dtype=np.int16)
    slots = np.full((N_CORES, T * 128), -1.0, dtype=np.float32)
    uvals = np.zeros((N_CORES, T * 128), dtype=np.float32)
    for c in range(N_CORES):
        es, eslot, ch, w, ue = per_core[c]
        keys = list(zip(w // WG, ch, w))
        run_start = {}
        for i2, k in enumerate(keys):
            if k not in run_start:
                run_start[k] = i2
        run_len = counts[c]
        out_pos = 0
        for g, segs in sched:
            for chp in range(N_CHUNKS):
                for wseg, nt in segs[chp]:
                    cnt = int(run_len[chp, wseg])
                    if cnt > 0:
                        i0 = run_start[(g, chp, wseg)]
                        sl = slice(i0, i0 + cnt)
                        local = (es[sl] - chp * CHUNK).astype(np.int16)
                        idx16[c, out_pos : out_pos + cnt] = local
                        slots[c, out_pos : out_pos + cnt] = eslot[sl].astype(
                            np.float32
                        )
                        uvals[c, out_pos : out_pos + cnt] = ue[sl].astype(np.float32)
                    out_pos += nt * 128
        assert out_pos == T * 128

    return u, n_tiles, sched, T, idx16, slots, uvals, perm


def _raw_gather(gp, mybir, out_ap, in_ap, idxs_ap, num_idxs, elem_size, elem_step):
    """dma_gather (non-transpose, HBM source) without the 256B-multiple
    elem restriction: elem_size may be any size as long as the source row
    STRIDE (elem_step) is a 256B multiple. Mirrors bass.BassGpSimd.dma_gather."""
    import concourse.ap_utils as ap_utils

    assert idxs_ap.dtype == mybir.dt.int16
    assert in_ap.dtype == out_ap.dtype
    stride_bytes = elem_step * mybir.dt.size(in_ap.dtype)
    assert stride_bytes % 256 == 0 and stride_bytes // 256 < 256
    assert ap_utils.ap_is_contiguous(in_ap.ap[1:])
    assert ap_utils.ap_is_contiguous(out_ap.ap[1:])
    assert ap_utils.ap_is_contiguous(idxs_ap.ap[1:])
    assert in_ap.ap[-1][1] == out_ap.ap[-1][1] == elem_size
    assert in_ap.ap[0][0] == elem_step
    _in_ap = gp.lower_ap_dma(in_ap, for_custom_bir_dma=True)
    _idxs_ap = gp.lower_ap(idxs_ap)
    _out_ap = gp.lower_ap(out_ap)
    return gp.add_instruction(
        mybir.InstDMAGatherAnt(
            name=gp.bass.get_next_instruction_name(),
            ins=[*_in_ap, _idxs_ap, gp.lower_val_access(gp.to_reg(num_idxs))],
            outs=[_out_ap],
            transpose=False,
            num_idxs=num_idxs,
            elem_size=elem_size,
            stride_bytes_256=stride_bytes // 256,
            gen_mode=0,
            single_packet=False,
            queue_num=0,
            sbuf_tokens_per_rank=0,
            sbuf_free_dim_per_rank=0,
            sbuf_free_dim_pad_per_rank=0,
            sbuf_byte_offset=0,
        )
    )


def _build_program(T, sched, has_bias):
    import concourse.bacc as bacc
    import concourse.mybir as mybir
    from concourse import tile

    f32 = mybir.dt.float32
    bf16 = mybir.dt.bfloat16
    fp8 = mybir.dt.float8e4

    nc = bacc.Bacc(
        "TRN2",
        target_bir_lowering=False,
        debug=False,
        enable_asserts=True,
        num_devices=N_CORES,
    )

    y_d = nc.dram_tensor("y8", [NPAD, YSTRIDE], fp8, kind="ExternalInput").ap()
    idx_d = nc.dram_tensor("idx16", [128, T * 8], mybir.dt.int16, kind="ExternalInput").ap()
    slots_d = nc.dram_tensor("slots", [128, T], f32, kind="ExternalInput").ap()
    uvals_d = nc.dram_tensor("uvals", [128, T], f32, kind="ExternalInput").ap()
    iota_d = nc.dram_tensor("iota", [128, 128], bf16, kind="ExternalInput").ap()
    ident_d = nc.dram_tensor("ident", [128, 128], bf16, kind="ExternalInput").ap()
    pidx_d = nc.dram_tensor("pidx", [128, 1], f32, kind="ExternalInput").ap()
    u2w_d = nc.dram_tensor("u2w", [128, WN], f32, kind="ExternalInput").ap()
    xsnm_d = nc.dram_tensor("xsnm", [128, WN * F], bf16, kind="ExternalInput").ap()
    # with a nonzero bias the self-loop seed needs plain x (not x+b)
    xnm_d = (nc.dram_tensor("xnm", [128, WN * F], bf16, kind="ExternalInput").ap()
             if has_bias else xsnm_d)
    w_d = nc.dram_tensor("W", [F, F], f32, kind="ExternalInput").ap()
    out_d = nc.dram_tensor("outT", [128, S], bf16, kind="ExternalOutput").ap()

    # last chunk with tiles, per window (for matmul stop flags); -1 = none
    last_ch = {}
    for g, segs in sched:
        for ch in range(N_CHUNKS):
            for w, nt in segs[ch]:
                last_ch[w] = ch

    with tile.TileContext(nc) as tc:
        with (
            tc.tile_pool(name="const", bufs=1) as const_p,
            tc.tile_pool(name="acc", bufs=1) as acc_p,
            tc.tile_pool(name="msgs", bufs=5) as msgs_p,
            tc.tile_pool(name="oh", bufs=4) as oh_p,
            tc.tile_pool(name="diag", bufs=4) as diag_p,
            tc.tile_pool(name="psum", bufs=8, space="PSUM") as psum_p,
            tc.tile_pool(name="fin", bufs=3) as fin_p,
        ):
            idx_sb = const_p.tile([128, T * 8], mybir.dt.int16)
            slots_sb = const_p.tile([128, T], f32)
            uvals_sb = const_p.tile([128, T], f32)
            iota_sb = const_p.tile([128, 128], bf16)
            ident_sb = const_p.tile([128, 128], bf16)
            pidx_sb = const_p.tile([128, 1], f32)
            u2w_sb = const_p.tile([128, WN], f32)
            xsnm_sb = const_p.tile([128, WN * F], bf16)
            xnm_sb = (const_p.tile([128, WN * F], bf16) if has_bias else xsnm_sb)
            w_sb = const_p.tile([F, F], f32)
            acc = acc_p.tile([128, S], f32)

            # consts needed by group-0 compute load first; W (tail-only)
            # is deferred into the loop so it doesn't delay the first gathers
            nc.sync.dma_start(idx_sb[:], idx_d[:])
            nc.sync.dma_start(slots_sb[:], slots_d[:])
            nc.sync.dma_start(uvals_sb[:], uvals_d[:])
            nc.sync.dma_start(iota_sb[:], iota_d[:])
            nc.sync.dma_start(ident_sb[:], ident_d[:])
            nc.sync.dma_start(pidx_sb[:], pidx_d[:])
            nc.sync.dma_start(u2w_sb[:], u2w_d[:])
            nc.sync.dma_start(xsnm_sb[:], xsnm_d[:])
            if has_bias:
                nc.sync.dma_start(xnm_sb[:], xnm_d[:])

            SL = 512

            def emit_tail(s0, n):
                # out^T chunk = W^T @ acc + xs (xs seeded via identity)
                pf = psum_p.tile([128, SL], f32, tag="psum", name=f"pf_{s0}")
                for j in range((n + 127) // 128):
                    w = s0 // 128 + j
                    nc.tensor.matmul(
                        pf[:, j * 128 : j * 128 + 128],
                        lhsT=xsnm_sb[:, w * F : (w + 1) * F],
                        rhs=ident_sb[:],
                        start=(j == 0),
                        stop=False,
                    )
                nc.tensor.matmul(pf[:, :n], lhsT=w_sb[:], rhs=acc[:, s0 : s0 + n],
                                 start=False, stop=True)
                ot = fin_p.tile([128, SL], bf16, tag="ot")
                nc.scalar.copy(out=ot[:, :n], in_=pf[:, :n])
                nc.sync.dma_start(out_d[:, s0 : s0 + n], ot[:, :n])

            g_tile = 0   # global tile cursor
            tail_s0 = 0  # next output chunk to emit
            for g, segs in sched:
                ws = list(range(g * WG, min((g + 1) * WG, WN)))
                nbank = (len(ws) + 3) // 4
                banks = [psum_p.tile([128, 512], f32, tag="psum", name=f"ps_g{g}_{b}")
                         for b in range(nbank)]

                def wslice(w):
                    wl = w - g * WG
                    return banks[wl // 4][:, (wl % 4) * 128 : (wl % 4) * 128 + 128]

                # seed each window's PSUM quarter with the self-loop term
                # u^2*x via a diagonal rhs: diag[p, j] = (j == p) * u^2_p.
                # start=True zeroes the whole 2KB bank, so only the FIRST
                # matmul touching a bank may set it; later seeds add onto
                # the zeroed bank with start=False.
                for w in ws:
                    wl = w - g * WG
                    dg = diag_p.tile([128, 128], bf16, tag="diag")
                    nc.vector.tensor_scalar(
                        dg[:],
                        iota_sb[:],
                        pidx_sb[:, 0:1],
                        u2w_sb[:, w : w + 1],
                        mybir.AluOpType.is_equal,
                        mybir.AluOpType.mult,
                    )
                    nc.tensor.matmul(
                        wslice(w),
                        lhsT=xnm_sb[:, w * F : (w + 1) * F],
                        rhs=dg[:],
                        start=(wl % 4 == 0),
                        stop=w not in last_ch,
                    )
                for ch in range(N_CHUNKS):
                    seg_tiles = sum(nt for (_, nt) in segs[ch])
                    if seg_tiles == 0:
                        continue
                    flat = [(wseg, k, nt) for wseg, nt in segs[ch]
                            for k in range(nt)]
                    # split into bounded sub-gathers so msgs buffers stay
                    # small enough to multi-buffer across groups
                    GMAX = 48
                    for sub0 in range(0, seg_tiles, GMAX):
                        sub = flat[sub0 : sub0 + GMAX]
                        ntile = len(sub)
                        n_idx = ntile * 128
                        base = g_tile + sub0
                        msgs = msgs_p.tile([128, ntile * 128], fp8, tag="msgs")
                        m3 = msgs[:].rearrange("p (b f) -> p b f", f=F)
                        _raw_gather(
                            nc.gpsimd, mybir, m3,
                            y_d[ch * CHUNK : (ch + 1) * CHUNK, 0:F],
                            idx_sb[:, base * 8 : base * 8 + n_idx // 16],
                            n_idx, F, YSTRIDE,
                        )
                        # one-hots in 8-tile super-tiles so the tile framework
                        # batches buffer-reuse waits (PE consumes in-order)
                        OHB = 8
                        oh_sup = None
                        for ti, (wseg, k, nt) in enumerate(sub):
                            ob = ti % OHB
                            if ob == 0:
                                oh_sup = oh_p.tile([128, OHB * 128], bf16)
                            gt = base + ti
                            # oh[e, j] = (iota_j == slot_e) * u[dst_e]
                            nc.vector.tensor_scalar(
                                oh_sup[:, ob * 128 : ob * 128 + 128],
                                iota_sb[:],
                                slots_sb[:, gt : gt + 1],
                                uvals_sb[:, gt : gt + 1],
                                mybir.AluOpType.is_equal,
                                mybir.AluOpType.mult,
                            )
                            nc.tensor.matmul(
                                wslice(wseg),
                                lhsT=msgs[:, ti * 128 : (ti + 1) * 128],
                                rhs=oh_sup[:, ob * 128 : ob * 128 + 128],
                                start=False,
                                stop=(last_ch[wseg] == ch and k == nt - 1),
                            )
                    g_tile += seg_tiles
                if g == 0:
                    # tail-only const: queued behind group 0's gathers so it
                    # doesn't delay the pipeline start, but emitted before
                    # the first tail chunk reads it
                    nc.sync.dma_start(w_sb[:], w_d[:])
                # flush: Act copies PSUM banks into the (write-once) acc
                for b in range(nbank):
                    c0 = (g * WG + b * 4) * 128
                    ncols = min(512, S - c0)
                    nc.scalar.copy(out=acc[:, c0 : c0 + ncols], in_=banks[b][:, :ncols])
                # emit output chunks whose acc columns are fully flushed
                flushed = min((g + 1) * WG, WN) * 128
                while tail_s0 < S and tail_s0 + min(SL, S - tail_s0) <= flushed:
                    n = min(SL, S - tail_s0)
                    emit_tail(tail_s0, n)
                    tail_s0 += n
            assert g_tile == T
            assert tail_s0 == S

    nc.compile()
    return nc


_PROGRAM_CACHE = {}


def _get_program(T, sched, has_bias):
    key = (T, has_bias,
           tuple((g, tuple(tuple(seg) for seg in segs)) for g, segs in sched))
    if key not in _PROGRAM_CACHE:
        _PROGRAM_CACHE[key] = _build_program(T, sched, has_bias)
    return _PROGRAM_CACHE[key]


def _prepare(x, edge_index, W, b):
    x = np.asarray(x, dtype=np.float32)
    edge_index = np.asarray(edge_index)
    W = np.asarray(W, dtype=np.float32)
    b = np.asarray(b, dtype=np.float32)

    u, n_tiles, sched, T, idx16, slots, uvals, perm = _host_plan(edge_index)

    import ml_dtypes
    bf = ml_dtypes.bfloat16
    f8 = ml_dtypes.float8_e4m3
    y8 = np.zeros((NPAD, YSTRIDE), dtype=f8)
    y8[:N_NODES, :F] = (u[:, None] * x).astype(f8)

    iota = np.tile(np.arange(128, dtype=np.float32), (128, 1)).astype(bf)
    ident = np.eye(128, dtype=np.float32).astype(bf)

    u_ext = np.concatenate([u, [0.0]]).astype(np.float32)
    x_ext = np.concatenate([x, np.zeros((1, F), np.float32)], axis=0)
    xs_ext = x_ext + b[None, :]
    has_bias = bool(np.any(b != 0))
    pidx = np.arange(128, dtype=np.float32).reshape(128, 1)

    in_maps = []
    for c in range(N_CORES):
        rows = perm[c]
        idx_c = np.tile(idx16[c].reshape(-1, 16).T, (8, 1)).copy()  # [128, T*8]
        slots_c = slots[c].reshape(T, 128).T.copy()
        uvals_c = uvals[c].reshape(T, 128).T.copy()
        # node-major per-window tiles: [slot-partition, window, feature]
        xsnm = xs_ext[rows].astype(bf).reshape(WN, 128, F).transpose(1, 0, 2)
        u2w = (u_ext[rows] ** 2).astype(np.float32).reshape(WN, 128).T
        im = {
            "y8": y8,
            "idx16": idx_c,
            "slots": slots_c.astype(np.float32),
            "uvals": uvals_c.astype(np.float32),
            "iota": iota,
            "ident": ident,
            "pidx": pidx,
            "u2w": np.ascontiguousarray(u2w),
            "xsnm": np.ascontiguousarray(xsnm.reshape(128, WN * F)),
            "W": W,
        }
        if has_bias:
            xnm = x_ext[rows].astype(bf).reshape(WN, 128, F).transpose(1, 0, 2)
            im["xnm"] = np.ascontiguousarray(xnm.reshape(128, WN * F))
        in_maps.append(im)

    nc = _get_program(T, sched, has_bias)
    global _LAST_PERM
    _LAST_PERM = perm
    return nc, in_maps


_LAST_PERM = None


def _unshard(results, perm=None):
    if perm is None:
        perm = _LAST_PERM
    out = np.empty((N_NODES, F), dtype=np.float32)
    for c in range(N_CORES):
        rows = perm[c]
        valid = rows >= 0
        out[rows[valid]] = results[c]["outT"].T.astype(np.float32)[valid]
    return out


def kernel(x, edge_index, W, b):
    from concourse.bass_utils import run_bass_kernel_spmd

    nc, in_maps = _prepare(x, edge_index, W, b)
    res = run_bass_kernel_spmd(nc, in_maps, list(range(N_CORES)))
    return _unshard(res.results)


if __name__ == "__main__":
    rng = np.random.default_rng(0)
    x = rng.standard_normal((N_NODES, F), dtype=np.float32)
    ei = rng.integers(0, N_NODES, size=(2, 1600000)).astype(np.int64)
    W = rng.standard_normal((F, F), dtype=np.float32) / np.sqrt(F)
    b = np.zeros(F, dtype=np.float32)
    out = kernel(x=x, edge_index=ei, W=W, b=b)
    print(out.shape, out.dtype)
